# revision 23
# baseline (speedup 1.0000x reference)
"""Eq2to2 equivariant layer (Maron et al. 2-to-2 basis, 15 ops) as a Trainium2
Bass/Tile kernel, data-parallel over the batch axis N across 8 NeuronCores.

Math: the 15-basis contraction collapses to
  out[n,s] = sum_d C9[d,s]*x[n,d] + sum_d C10[d,s]*x[n,d]^T
           + Row[n,s,i] (bcast over j) + Col[n,s,j] (bcast over i)
           + delta_ij * DiagT[n,s,i] + Const[n,s] + bias[s] + delta_ij*diag_bias[s]
where Row/Col/DiagT/Const are small contractions of rowsum/colsum/diag/tot stats.

Layout: each core takes 4 n's -> 128 SBUF partitions = (nq, d). Grids are flat
in the free dim (16384 f32 per partition). The x^T einsum needs no data
movement: the matmul moving operand reads the grid through a transposed
strided AP ([[1,4],[128,128]]) within each partition.
"""

import sys

import numpy as np

if "/opt/trn_rl_repo" not in sys.path:
    sys.path.insert(0, "/opt/trn_rl_repo")

N, D, S, B, M = 32, 32, 32, 15, 128
NCORES = 8
NPC = N // NCORES          # n's per core = 4
P = 128                    # partitions
FREE = M * M               # 16384
CHUNK = 512                # psum bank (f32)
NCHUNK = FREE // CHUNK     # 32
OUTW = 2048                # out staging width (4 chunks)
NLOAD = 8                  # xa load slices
SL = FREE // NLOAD         # 2048 elements (16 i-rows) per load slice

_cache: dict = {}

# float32r: single-pass fp32 matmul (TF32-like mantissa on HW, 4x faster than
# exact fp32 for free-dim >= 256). Flip to False to fall back to exact fp32.
USE_F32R = True
ACT_CHUNKS = 16  # of the 32 chunks, how many get ACT-path assembly


def _build_program(repeat=1):
    import concourse.bass as bass
    import concourse.tile as tile
    from concourse import bacc, mybir

    f32 = mybir.dt.float32
    f32r = mybir.dt.float32r
    nc = bacc.Bacc("TRN2", target_bir_lowering=False, debug=False)

    xr_d = nc.dram_tensor("xr", [P, FREE], f32, kind="ExternalInput")
    wm_d = nc.dram_tensor("wmats", [15, P, P], f32, kind="ExternalInput")
    bc_d = nc.dram_tensor("bcols", [P, 2], f32, kind="ExternalInput")
    out_d = nc.dram_tensor("outr", [P, FREE], f32, kind="ExternalOutput")

    ADD = mybir.AluOpType.add
    IDENT = mybir.ActivationFunctionType.Identity

    with tile.TileContext(nc) as tc:
        with (
            tc.tile_pool(name="big", bufs=1) as big,
            tc.tile_pool(name="cst", bufs=1) as cst,
            tc.tile_pool(name="aux", bufs=1) as aux,
            tc.tile_pool(name="ot", bufs=3) as otp,
            tc.tile_pool(name="pm", bufs=6, space="PSUM") as pmp,
            tc.tile_pool(name="pa", bufs=1, space="PSUM") as pap,
        ):
          for _rep in range(repeat):
            # ---- constants ----
            # (loads bitcast to f32r so the verifier accepts f32r matmul use;
            #  host pre-rounds the data to fp32r precision)
            ldt = f32r if USE_F32R else f32
            wm = cst.tile([P, 15, P], f32)
            nc.sync.dma_start(out=wm[:].bitcast(ldt),
                              in_=wm_d[:].rearrange("w k m -> k w m").bitcast(ldt))
            bc = cst.tile([P, 2], f32)
            nc.sync.dma_start(out=bc[:], in_=bc_d[:])

            W = lambda idx: wm[:, idx, :]
            (W_X, W_XT, W_ROW_CS, W_ROW_RS, W_ROW_DG, W_COL_CS, W_COL_RS,
             W_COL_DG, W_DIA_DG, W_DIA_RS, W_DIA_CS, W_SD_SD, W_SD_TOT,
             W_SC_SD, W_SC_TOT) = range(15)

            # ---- stats tiles ----
            rowsum = aux.tile([P, M], f32)   # rowsum[p, i] = sum_j x[p, i, j]
            colsum = aux.tile([P, M], f32)   # colsum[p, j] = sum_i x[p, i, j]
            diagx = aux.tile([P, M], f32)    # diag[p, i] = x[p, i, i]
            sd = aux.tile([P, 1], f32)       # sum of diag
            tot = aux.tile([P, 1], f32)      # total sum
            pacc = aux.tile([P, SL // 2], f32)   # colsum accumulator (Pool, slices 0-4)
            ptm2 = aux.tile([P, SL // 2], f32)   # per-slice pair sum (Pool)
            dacc = aux.tile([P, SL // 2], f32)   # colsum accumulator (DVE, slices 5-7)
            dtm2 = aux.tile([P, SL // 2], f32)   # per-slice pair sum (DVE)

            # ---- load x rows; stats per slice overlap the loads ----
            xa = big.tile([P, FREE], f32)
            xa_ap = xa[:]

            def ap(offset, dims):
                return bass.AP(
                    tensor=xa_ap.tensor,
                    offset=xa_ap.offset + offset,
                    ap=[list(xa_ap.ap[0])] + dims,
                )

            IPS = SL // M  # i-rows per slice = 16
            for t in range(NLOAD):
                sl = slice(t * SL, (t + 1) * SL)
                nc.sync.dma_start(out=xa[:, sl].bitcast(ldt),
                                  in_=xr_d[:, sl].bitcast(ldt))
                # rowsum of this slice's 16 i-rows (DVE)
                nc.vector.reduce_sum(
                    out=rowsum[:, t * IPS:(t + 1) * IPS],
                    in_=ap(t * SL, [[M, IPS], [1, M]]),
                    axis=mybir.AxisListType.X,
                )
                # colsum partials: fold each slice's 16 i-rows to 8 rows;
                # slices 0-4 tree-merged on GPSIMD, 5-7 on DVE (late slices,
                # short tail after the last load lands)
                if t < 5:
                    eng, acc, tmp = nc.gpsimd, pacc, ptm2
                else:
                    eng, acc, tmp = nc.vector, dacc, dtm2
                dst = acc if t in (0, 5) else tmp
                eng.tensor_tensor(out=dst[:], in0=xa[:, t * SL: t * SL + SL // 2],
                                  in1=xa[:, t * SL + SL // 2:(t + 1) * SL], op=ADD)
                if t not in (0, 5):
                    eng.tensor_tensor(out=acc[:], in0=acc[:], in1=tmp[:], op=ADD)
            # merge accumulators + fold 8 i-rows into colsum (DVE, tiny)
            nc.vector.tensor_tensor(out=pacc[:], in0=pacc[:], in1=dacc[:], op=ADD)
            nc.vector.tensor_tensor(out=pacc[:, 0:SL // 4], in0=pacc[:, 0:SL // 4],
                                    in1=pacc[:, SL // 4:SL // 2], op=ADD)
            nc.vector.tensor_tensor(out=pacc[:, 0:2 * M], in0=pacc[:, 0:2 * M],
                                    in1=pacc[:, 2 * M:4 * M], op=ADD)
            nc.vector.tensor_tensor(out=colsum[:], in0=pacc[:, 0:M],
                                    in1=pacc[:, M:2 * M], op=ADD)
            # diag: one strided copy (f = 129*i), then scalars
            nc.vector.tensor_copy(out=diagx[:], in_=ap(0, [[M + 1, M]]))
            nc.vector.reduce_sum(out=sd[:], in_=diagx[:], axis=mybir.AxisListType.X)
            nc.vector.reduce_sum(out=tot[:], in_=rowsum[:], axis=mybir.AxisListType.X)

            # ---- aux contractions over d (partition dim) on the PE ----
            pa = pap.tile([P, CHUNK], f32)  # sections: row | col | diag | scal
            mm = nc.tensor.matmul
            mm(pa[:, 0:M], W(W_ROW_CS), colsum[:], start=True, stop=False)
            mm(pa[:, 0:M], W(W_ROW_RS), rowsum[:], start=False, stop=False)
            mm(pa[:, 0:M], W(W_ROW_DG), diagx[:], start=False, stop=True)

            mm(pa[:, M:2 * M], W(W_COL_CS), colsum[:], start=True, stop=False)
            mm(pa[:, M:2 * M], W(W_COL_RS), rowsum[:], start=False, stop=False)
            mm(pa[:, M:2 * M], W(W_COL_DG), diagx[:], start=False, stop=True)

            mm(pa[:, 2 * M:3 * M], W(W_DIA_DG), diagx[:], start=True, stop=False)
            mm(pa[:, 2 * M:3 * M], W(W_DIA_RS), rowsum[:], start=False, stop=False)
            mm(pa[:, 2 * M:3 * M], W(W_DIA_CS), colsum[:], start=False, stop=True)

            mm(pa[:, 3 * M:3 * M + 1], W(W_SD_SD), sd[:], start=True, stop=False)
            mm(pa[:, 3 * M:3 * M + 1], W(W_SD_TOT), tot[:], start=False, stop=True)
            mm(pa[:, 3 * M + 1:3 * M + 2], W(W_SC_SD), sd[:], start=True, stop=False)
            mm(pa[:, 3 * M + 1:3 * M + 2], W(W_SC_TOT), tot[:], start=False, stop=True)

            # fold constants: RowF = Row + Const + bias; DiagF = DiagT + DiagConst + diag_bias
            rowf = aux.tile([P, M], f32)
            colf = aux.tile([P, M], f32)
            diaf = aux.tile([P, M], f32)
            nc.vector.tensor_scalar(out=rowf[:], in0=pa[:, 0:M],
                                    scalar1=pa[:, 3 * M + 1:3 * M + 2],
                                    scalar2=bc[:, 0:1], op0=ADD, op1=ADD)
            nc.vector.tensor_copy(out=colf[:], in_=pa[:, M:2 * M])
            nc.vector.tensor_scalar(out=diaf[:], in0=pa[:, 2 * M:3 * M],
                                    scalar1=pa[:, 3 * M:3 * M + 1],
                                    scalar2=bc[:, 1:2], op0=ADD, op1=ADD)

            # ---- main einsum + assembly, streamed in 512-wide chunks ----
            cast = (lambda a: a.bitcast(f32r)) if USE_F32R else (lambda a: a)
            for g in range(NCHUNK // 4):  # output-staging groups of 4 chunks
                ot = otp.tile([P, OUTW], f32)
                for cc in range(4):
                    c = g * 4 + cc
                    i0 = 4 * c
                    pm = pmp.tile([P, CHUNK], f32, tag="pm")
                    # C9 term: contiguous grid chunk (rows i0..i0+3)
                    mm(pm[:], cast(W(W_X)), cast(xa[:, c * CHUNK:(c + 1) * CHUNK]),
                       start=True, stop=False)
                    # C10 term: transposed read of the same output window
                    mm(pm[:], cast(W(W_XT)), cast(ap(i0, [[1, 4], [M, M]])),
                       start=False, stop=True)
                    # out = (psum + RowF[i]) + ColF[j]
                    if (c % 2 == 0) and ACT_CHUNKS > 0:
                        # ACT path: psum + RowF via activation bias; ColF via
                        # one GPSIMD add with a broadcast (stride-0) AP
                        for q in range(4):
                            nc.scalar.activation(
                                out=ot[:, cc * CHUNK + q * M: cc * CHUNK + (q + 1) * M],
                                in_=pm[:, q * M:(q + 1) * M],
                                func=IDENT,
                                bias=rowf[:, i0 + q:i0 + q + 1],
                            )
                        cfb = bass.AP(tensor=colf[:].tensor, offset=colf[:].offset,
                                      ap=[list(colf[:].ap[0]), [0, 4], [1, M]])
                        otv = ot[:, cc * CHUNK:(cc + 1) * CHUNK].rearrange(
                            "p (i j) -> p i j", i=4)
                        nc.gpsimd.tensor_tensor(out=otv, in0=otv, in1=cfb, op=ADD)
                    else:
                        for q in range(4):
                            nc.vector.scalar_tensor_tensor(
                                out=ot[:, cc * CHUNK + q * M: cc * CHUNK + (q + 1) * M],
                                in0=pm[:, q * M:(q + 1) * M],
                                scalar=rowf[:, i0 + q:i0 + q + 1],
                                in1=colf[:],
                                op0=ADD, op1=ADD,
                            )
                    # diagonal add: positions f_local = cc*512 + i0 + 129*q
                    ot_ap = ot[:]
                    dview = bass.AP(
                        tensor=ot_ap.tensor,
                        offset=ot_ap.offset + cc * CHUNK + i0,
                        ap=[list(ot_ap.ap[0]), [M + 1, 4]],
                    )
                    nc.vector.tensor_tensor(out=dview, in0=dview,
                                            in1=diaf[:, i0:i0 + 4], op=ADD)
                nc.sync.dma_start(out=out_d[:, g * OUTW:(g + 1) * OUTW], in_=ot[:])

    nc.compile()
    return nc


def _get_nc():
    if "nc" not in _cache:
        _cache["nc"] = _build_program()
    return _cache["nc"]


def _host_prep(coefs, bias, diag_bias):
    m = float(M)
    C = np.asarray(coefs, dtype=np.float32)
    eye4 = np.eye(NPC, dtype=np.float32)

    def bd(b, scale=1.0):
        return np.kron(eye4, C[:, :, b] * np.float32(scale))

    wmats = np.stack([
        bd(9),              # W_X
        bd(10),             # W_XT
        bd(5, 1 / m),       # W_ROW_CS
        bd(6, 1 / m),       # W_ROW_RS
        bd(11),             # W_ROW_DG
        bd(7, 1 / m),       # W_COL_CS
        bd(8, 1 / m),       # W_COL_RS
        bd(12),             # W_COL_DG
        bd(0),              # W_DIA_DG
        bd(2, 1 / m),       # W_DIA_RS
        bd(3, 1 / m),       # W_DIA_CS
        bd(1, 1 / m),       # W_SD_SD
        bd(4, 1 / (m * m)),  # W_SD_TOT
        bd(13, 1 / m),      # W_SC_SD
        bd(14, 1 / (m * m)),  # W_SC_TOT
    ]).astype(np.float32)
    bcols = np.stack([
        np.tile(np.asarray(bias, np.float32).reshape(S), NPC),
        np.tile(np.asarray(diag_bias, np.float32).reshape(S), NPC),
    ], axis=1).astype(np.float32)
    return np.ascontiguousarray(wmats), np.ascontiguousarray(bcols)


def _round_f32r(a):
    # fp32r-representable = exact sum of two bf16s (what the PE's single-pass
    # fp32 mode assumes); ~2^-16 relative rounding.
    import ml_dtypes

    hi = a.astype(ml_dtypes.bfloat16).astype(np.float32)
    lo = (a - hi).astype(ml_dtypes.bfloat16).astype(np.float32)
    return hi + lo


def _in_maps(inputs, coefs, bias, diag_bias):
    x = np.ascontiguousarray(np.asarray(inputs, np.float32))
    wmats, bcols = _host_prep(coefs, bias, diag_bias)
    if USE_F32R:
        x = _round_f32r(x)
        wmats = _round_f32r(wmats)
    maps = []
    for i in range(NCORES):
        xr = x[i * NPC:(i + 1) * NPC].reshape(P, FREE)
        maps.append({"xr": np.ascontiguousarray(xr), "wmats": wmats, "bcols": bcols})
    return maps


def run(inputs, coefs, bias, diag_bias, **spmd_kwargs):
    """Run on the 8 NeuronCores; returns (output, BassKernelResults)."""
    global USE_F32R
    from concourse.bass_utils import run_bass_kernel_spmd

    nc = _get_nc()
    maps = _in_maps(inputs, coefs, bias, diag_bias)
    try:
        res = run_bass_kernel_spmd(nc, maps, list(range(NCORES)), **spmd_kwargs)
    except Exception:
        if not USE_F32R:
            raise
        # fall back to exact fp32 matmuls if fp32r fails to compile/run here
        USE_F32R = False
        _cache.clear()
        nc = _get_nc()
        maps = _in_maps(inputs, coefs, bias, diag_bias)
        res = run_bass_kernel_spmd(nc, maps, list(range(NCORES)), **spmd_kwargs)
    out = np.concatenate(
        [r["outr"].reshape(NPC, S, M, M) for r in res.results], axis=0
    )
    return np.ascontiguousarray(out.astype(np.float32)), res


def kernel(inputs, coefs, bias, diag_bias):
    out, _ = run(inputs, coefs, bias, diag_bias)
    return out
